# revision 4
# baseline (speedup 1.0000x reference)
"""Trainium2 Bass kernel for DCTEncoderLayer.

Computes, for rgb_images_batch [32, 3, 512, 512] f32:
  ycbcr' = 2*rgb_to_ycbcr(rgb) - 1                 (per-pixel 3x3 channel mix, linear + const)
  32x32 block DCT per channel, coefficients scaled by (2/32)*c_u*c_v,
  output [32, 3*1024, 16, 16] with the frequency axis sorted by |(v,u)|.

Strategy (pure data parallel over batch, 4 images per NeuronCore):
  The 2D DCT is separable: coeff = Cs @ block @ Cs.T with Cs[v,y] =
  cos((2y+1)v*pi/64) * c_v / 4.  The YCbCr channel mix is linear and is
  folded into the stage-1 weights (contraction runs over (channel, y)).
  Per (image, block-row) iteration on device:
    stage1:  t1[(c,v), x]   = W1m.T @ img[(c',y), x]      (one matmul, N=512)
    PE-transpose x-chunks:  tT[x', (g,c,v)]               (4 transposes)
    stage2:  out[(bc,u), (g,c,v)] = W2bd.T @ tT           (one matmul, N=384)
  All matmul operands use float32r (TF32-like, ~1e-4 rel err, 4x faster
  than fp32 on the PE).  The device writes raw [64, 128, 384] tiles;
  the host reassembles/permutes axes, applies the frequency sort and the
  constant DC offset of the affine YCbCr transform (only the Y channel
  has one: 2*y-1 -> DC shift of -32).
"""

import os
import sys

try:
    import concourse.bass  # noqa: F401
except ImportError:  # bare interpreter without the axon site paths
    sys.path.insert(0, "/opt/trn_rl_repo")

import numpy as np

import concourse.bacc as bacc
import concourse.bass as bass
import concourse.mybir as mybir
import concourse.tile as tile
from concourse.bass_utils import run_bass_kernel_spmd

F32 = mybir.dt.float32
F32R = mybir.dt.float32r

BS = 32            # DCT block size
N_CORES = 8
B_PER_CORE = 4     # batch images per core
NH = 16            # blocks per column (512/32)
ITERS = B_PER_CORE * NH  # 64 per core

_STATE = {}
LAST_RESULT = None  # BassKernelResults of the most recent run (for profiling)


def _dct_mat():
    """Cs[v, y] = cos((2y+1) v pi / 64) * c_v / 4  (f64)."""
    y = np.arange(BS)
    v = np.arange(BS)[:, None]
    c = np.cos((2 * y + 1) * v * np.pi / (2 * BS))
    c[0, :] *= 1.0 / np.sqrt(2.0)
    return c / 4.0


def _sort_idx():
    # must replicate the reference's argsort (default kind) exactly,
    # including its tie order for equal |(v,u)|
    mag = np.zeros((BS, BS), dtype=np.float64)
    for v in range(BS):
        for u in range(BS):
            mag[v, u] = np.linalg.norm(np.array([v, u], dtype=np.int64))
    return np.argsort(mag.reshape(-1))


def _constants():
    cs = _dct_mat()
    # rows (y', cb', cr') of the linear part of 2*rgb_to_ycbcr(rgb)-1, in (r,g,b)
    a2 = np.array(
        [
            [2 * 0.299, 2 * 0.587, 2 * 0.114],
            [2 * 0.564 * -0.299, 2 * 0.564 * -0.587, 2 * 0.564 * (1 - 0.114)],
            [2 * 0.713 * (1 - 0.299), 2 * 0.713 * -0.587, 2 * 0.713 * -0.114],
        ],
        np.float64,
    )
    w1 = np.zeros((96, 96))  # [(c', y), (c, v)]
    for cp in range(3):
        for c in range(3):
            w1[cp * 32 : (cp + 1) * 32, c * 32 : (c + 1) * 32] = a2[c, cp] * cs.T
    w2 = np.zeros((128, 128))  # [(bc, x'), (bc, u)]
    for bc in range(4):
        w2[bc * 32 : (bc + 1) * 32, bc * 32 : (bc + 1) * 32] = cs.T
    ident = np.eye(96)
    return (
        w1.astype(np.float32),
        w2.astype(np.float32),
        ident.astype(np.float32),
    )


def _build_program():
    nc = bacc.Bacc(trn_type="TRN2")
    x = nc.dram_tensor("x", [B_PER_CORE, 3, 512, 512], F32R, kind="ExternalInput")
    w1 = nc.dram_tensor("w1", [96, 96], F32R, kind="ExternalInput")
    w2 = nc.dram_tensor("w2", [128, 128], F32R, kind="ExternalInput")
    idn = nc.dram_tensor("idn", [96, 96], F32R, kind="ExternalInput")
    out = nc.dram_tensor("out", [ITERS, 128, 384], F32, kind="ExternalOutput")

    with tile.TileContext(nc) as tc:
        with (
            tc.tile_pool(name="const", bufs=1) as constp,
            tc.tile_pool(name="sb", bufs=3) as sb,
            tc.tile_pool(name="psA", bufs=2, space="PSUM") as psA,
            tc.tile_pool(name="psB", bufs=2, space="PSUM") as psB,
            tc.tile_pool(name="psC", bufs=2, space="PSUM") as psC,
        ):
            w1s = constp.tile([96, 96], F32R)
            w2s = constp.tile([128, 128], F32R)
            ids = constp.tile([96, 96], F32R)
            nc.sync.dma_start(w1s[:], w1[:])
            nc.sync.dma_start(w2s[:], w2[:])
            nc.sync.dma_start(ids[:], idn[:])

            for it in range(ITERS):
                b, br = it // NH, it % NH
                img = sb.tile([96, 512], F32R, tag="img")
                nc.sync.dma_start(
                    img[:],
                    x[b, :, br * 32 : (br + 1) * 32, :],
                )
                # stage 1: t1[(c,v), x] = W1m.T @ img
                t1p = psA.tile([96, 512], F32, tag="t1p")
                nc.tensor.matmul(t1p[:], w1s[:], img[:], start=True, stop=True)
                t1s = sb.tile([96, 512], F32R, tag="t1s")
                nc.vector.tensor_copy(t1s[:], t1p[:])
                # PE transpose of the 4 x-chunks: tT[x', (g, c, v)]
                tTp = psB.tile([128, 384], F32R, tag="tTp")
                for g in range(4):
                    nc.tensor.transpose(
                        tTp[:, 96 * g : 96 * (g + 1)],
                        t1s[:, 128 * g : 128 * (g + 1)],
                        ids[:],
                    )
                tTs = sb.tile([128, 384], F32R, tag="tTs")
                nc.scalar.copy(tTs[:], tTp[:])
                # stage 2: out2[(bc,u), (g,c,v)] = W2bd.T @ tT
                o2p = psC.tile([128, 384], F32, tag="o2p")
                nc.tensor.matmul(o2p[:], w2s[:], tTs[:], start=True, stop=True)
                osb = sb.tile([128, 384], F32, tag="osb")
                nc.scalar.copy(osb[:], o2p[:])
                nc.sync.dma_start(out[it], osb[:])

    nc.finalize()
    return nc


def _get_program():
    if "nc" not in _STATE:
        _STATE["nc"] = _build_program()
        _STATE["consts"] = _constants()
        _STATE["sort_idx"] = _sort_idx()
    return _STATE["nc"]


def kernel(**inputs):
    global LAST_RESULT
    rgb = np.ascontiguousarray(np.asarray(inputs["rgb_images_batch"], np.float32))
    assert rgb.shape == (N_CORES * B_PER_CORE, 3, 512, 512)
    nc = _get_program()
    w1, w2, ident = _STATE["consts"]
    sort_idx = _STATE["sort_idx"]

    in_maps = [
        {
            "x": rgb[c * B_PER_CORE : (c + 1) * B_PER_CORE],
            "w1": w1,
            "w2": w2,
            "idn": ident,
        }
        for c in range(N_CORES)
    ]
    trace = os.environ.get("KERNEL_TRACE", "0") == "1"
    res = run_bass_kernel_spmd(
        nc, in_maps, core_ids=list(range(N_CORES)), trace=trace
    )
    LAST_RESULT = res

    outs = []
    for c in range(N_CORES):
        dev = res.results[c]["out"]  # [64, 128, 384]
        a = dev.reshape(B_PER_CORE, NH, 4, 32, 4, 3, 32)  # b,br,bc,u,g,c,v
        a = a.transpose(0, 5, 6, 3, 1, 4, 2)  # b,c,v,u,br,g,bc
        a = np.ascontiguousarray(a).reshape(B_PER_CORE, 3, 1024, NH, NH)
        a = a[:, :, sort_idx, :, :]
        a[:, 0, 0, :, :] -= 32.0  # DC offset of the Y channel (2*y - 1)
        outs.append(a.reshape(B_PER_CORE, 3 * 1024, NH, NH))
    return np.concatenate(outs, axis=0)


# revision 5
# speedup vs baseline: 1.0205x; 1.0205x over previous
"""Trainium2 Bass kernel for DCTEncoderLayer.

Computes, for rgb_images_batch [32, 3, 512, 512] f32:
  ycbcr' = 2*rgb_to_ycbcr(rgb) - 1                 (per-pixel 3x3 channel mix, affine)
  32x32 block DCT per channel, coefficients scaled by (2/32)*c_u*c_v,
  output [32, 3*1024, 16, 16] with the frequency axis sorted by |(v,u)|.

Strategy (pure data parallel over batch, 4 images per NeuronCore):
  The 2D DCT is separable: coeff = Cs @ block @ Cs.T with Cs[v,y] =
  cos((2y+1)v*pi/64) * c_v / 4.  The YCbCr channel mix is linear and is
  folded into the stage-1 weights (contraction runs over (channel, y));
  feeding the device rgb-0.5 makes the affine offset exact (the shifted
  input has zero offset in every output channel).
  Per (image, block-row) iteration on device:
    stage1: t1[(c,v), x]       = W1m.T @ img[(c',y), x]     (matmul, N=512)
    stream_transpose (DVE):    tbt[(c,x'), (gx,v)]          (32x32 blockwise,
                               exactly what the block-diagonal stage-2 needs)
    round to f32r:             tbr = tbt                    (DVE/ACT copy)
    stage2: out[(c,u),(gx,v)]  = W2bd.T @ tbr               (matmul, N=512)
  Matmuls run in float32r (TF32-like, ~1e-4 rel err, 4x faster than fp32
  on the PE).  The device writes raw [64, 96, 512] tiles; the host
  reassembles/permutes axes and applies the frequency sort.
"""

import os
import sys

try:
    import concourse.bass  # noqa: F401
except ImportError:  # bare interpreter without the axon site paths
    sys.path.insert(0, "/opt/trn_rl_repo")

import numpy as np

import concourse.bacc as bacc
import concourse.bass as bass
import concourse.mybir as mybir
import concourse.tile as tile
from concourse.bass_utils import run_bass_kernel_spmd

F32 = mybir.dt.float32
F32R = mybir.dt.float32r

BS = 32            # DCT block size
N_CORES = 8
B_PER_CORE = 4     # batch images per core
NH = 16            # blocks per row/column (512/32)
ITERS = B_PER_CORE * NH  # 64 per core

_STATE = {}
LAST_RESULT = None  # BassKernelResults of the most recent run (for profiling)


def _dct_mat():
    """Cs[v, y] = cos((2y+1) v pi / 64) * c_v / 4  (f64)."""
    y = np.arange(BS)
    v = np.arange(BS)[:, None]
    c = np.cos((2 * y + 1) * v * np.pi / (2 * BS))
    c[0, :] *= 1.0 / np.sqrt(2.0)
    return c / 4.0


def _sort_idx():
    # must replicate the reference's argsort (default kind) exactly,
    # including its tie order for equal |(v,u)|
    mag = np.zeros((BS, BS), dtype=np.float64)
    for v in range(BS):
        for u in range(BS):
            mag[v, u] = np.linalg.norm(np.array([v, u], dtype=np.int64))
    return np.argsort(mag.reshape(-1))


def _constants():
    cs = _dct_mat()
    # rows (y', cb', cr') of the linear part of 2*rgb_to_ycbcr(rgb)-1, in (r,g,b)
    a2 = np.array(
        [
            [2 * 0.299, 2 * 0.587, 2 * 0.114],
            [2 * 0.564 * -0.299, 2 * 0.564 * -0.587, 2 * 0.564 * (1 - 0.114)],
            [2 * 0.713 * (1 - 0.299), 2 * 0.713 * -0.587, 2 * 0.713 * -0.114],
        ],
        np.float64,
    )
    w1 = np.zeros((96, 96))  # [(c', y), (c, v)]
    for cp in range(3):
        for c in range(3):
            w1[cp * 32 : (cp + 1) * 32, c * 32 : (c + 1) * 32] = a2[c, cp] * cs.T
    w2 = np.zeros((96, 96))  # [(c, x'), (c, u)] block diagonal over c
    for c in range(3):
        w2[c * 32 : (c + 1) * 32, c * 32 : (c + 1) * 32] = cs.T
    return w1.astype(np.float32), w2.astype(np.float32)


def _build_program():
    nc = bacc.Bacc(trn_type="TRN2")
    x = nc.dram_tensor("x", [B_PER_CORE, 3, 512, 512], F32R, kind="ExternalInput")
    w1 = nc.dram_tensor("w1", [96, 96], F32R, kind="ExternalInput")
    w2 = nc.dram_tensor("w2", [96, 96], F32R, kind="ExternalInput")
    out = nc.dram_tensor("out", [ITERS, 96, 512], F32, kind="ExternalOutput")

    with tile.TileContext(nc) as tc:
        with (
            tc.tile_pool(name="const", bufs=1) as constp,
            tc.tile_pool(name="sb", bufs=3) as sb,
            tc.tile_pool(name="psA", bufs=2, space="PSUM") as psA,
            tc.tile_pool(name="psB", bufs=2, space="PSUM") as psB,
        ):
            w1s = constp.tile([96, 96], F32R)
            w2s = constp.tile([96, 96], F32R)
            nc.sync.dma_start(w1s[:], w1[:])
            nc.sync.dma_start(w2s[:], w2[:])

            for it in range(ITERS):
                b, br = it // NH, it % NH
                img = sb.tile([96, 512], F32R, tag="img")
                nc.sync.dma_start(img[:], x[b, :, br * 32 : (br + 1) * 32, :])
                # stage 1: t1[(c,v), x] = W1m.T @ img
                t1p = psA.tile([96, 512], F32, tag="t1p")
                nc.tensor.matmul(t1p[:], w1s[:], img[:], start=True, stop=True)
                # 32x32 blockwise transpose: tbt[(c,x'), (gx,v)]
                tbt = sb.tile([96, 512], F32, tag="tbt")
                nc.vector.transpose(tbt[:], t1p[:])
                # round to f32r for stage 2 (alternate engines to balance load)
                tbr = sb.tile([96, 512], F32R, tag="tbr")
                if it % 2 == 0:
                    nc.vector.tensor_copy(tbr[:], tbt[:])
                else:
                    nc.scalar.copy(tbr[:], tbt[:])
                # stage 2: out2[(c,u), (gx,v)] = W2bd.T @ tbt
                o2p = psB.tile([96, 512], F32, tag="o2p")
                nc.tensor.matmul(o2p[:], w2s[:], tbr[:], start=True, stop=True)
                osb = sb.tile([96, 512], F32, tag="osb")
                nc.scalar.copy(osb[:], o2p[:])
                nc.sync.dma_start(out[it], osb[:])

    nc.finalize()
    return nc


def _get_program():
    if "nc" not in _STATE:
        _STATE["nc"] = _build_program()
        _STATE["consts"] = _constants()
        _STATE["sort_idx"] = _sort_idx()
    return _STATE["nc"]


def kernel(**inputs):
    global LAST_RESULT
    rgb = np.asarray(inputs["rgb_images_batch"], np.float32)
    assert rgb.shape == (N_CORES * B_PER_CORE, 3, 512, 512)
    # centering makes the YCbCr affine offset vanish (row sums of the cb/cr
    # mix are 0 and the y row sums to 2 -> offset 2*0.5-1=0 for every channel)
    xs = np.ascontiguousarray(rgb - np.float32(0.5))
    nc = _get_program()
    w1, w2 = _STATE["consts"]
    sort_idx = _STATE["sort_idx"]

    in_maps = [
        {"x": xs[c * B_PER_CORE : (c + 1) * B_PER_CORE], "w1": w1, "w2": w2}
        for c in range(N_CORES)
    ]
    trace = os.environ.get("KERNEL_TRACE", "0") == "1"
    res = run_bass_kernel_spmd(
        nc, in_maps, core_ids=list(range(N_CORES)), trace=trace
    )
    LAST_RESULT = res

    outs = []
    for c in range(N_CORES):
        dev = res.results[c]["out"]  # [64, 96, 512]
        a = dev.reshape(B_PER_CORE, NH, 3, 32, NH, 32)  # b, br, c, u, gx, v
        a = a.transpose(0, 2, 5, 3, 1, 4)  # b, c, v, u, br, gx
        a = np.ascontiguousarray(a).reshape(B_PER_CORE, 3, 1024, NH, NH)
        a = a[:, :, sort_idx, :, :]
        outs.append(a.reshape(B_PER_CORE, 3 * 1024, NH, NH))
    return np.concatenate(outs, axis=0)
